# revision 1
# baseline (speedup 1.0000x reference)
"""Trainium2 Bass kernel for nn_DifferentiableCDF (soft Gaussian histogram -> CDF).

Algorithm (per core, data-parallel over pixels; 12 units x 8192 px each):
  u = 255*x in bin units; hi = floor(u/16) (16-bin block), d = u/16 - hi in [0,1].
  Gaussian weight for bin j = 16*(hi-1) + w (w in [W_LO, W_HI)) is
    exp(-ALPHAP*(d + c_w)^2),  c_w = (16-w)/16,  ALPHAP = 256/(255*sigma)^2
  (truncated support: |u - j| > ~10 bins contributes < 1e-6 relative).
  Device pipeline: DVE builds dm = d + c_w per column (bf16 4x tensor_scalar)
  and arg = dm*dm (bf16 2x tensor_tensor), ACT applies one bulk Exp per pixel
  group into fp16 weights, and the TensorE scatters each 128-pixel chunk with
  a 16-wide one-hot-of-hi matmul accumulating into a per-(unit, hi) PSUM
  table.  Host folds the 16 x 34-column block tables into the 256-bin
  histogram (the only cross-core reduction), normalizes, and cumsums.
"""
import sys
if "/opt/trn_rl_repo" not in sys.path:
    sys.path.insert(0, "/opt/trn_rl_repo")

import numpy as np
from concourse import bacc, tile
from concourse.bass_utils import run_bass_kernel_spmd
import concourse.mybir as mybir

# ---- problem constants (hardcoded per spec) ----
B, C, H, W = 4, 3, 256, 256
UNITS = B * C                  # 12 independent histograms
NPIX = H * W                   # 65536 pixels per unit
NCORES = 8
PIX_PER_CORE = NPIX // NCORES  # 8192 pixels per unit per core
CHUNKS_PER_UNIT = PIX_PER_CORE // 128  # 64
NCHUNK = UNITS * CHUNKS_PER_UNIT       # 768 chunks of 128 pixels
SIGMA = 0.01
BINS = 256
ALPHAP = 256.0 / (255.0 * SIGMA) ** 2  # 39.369...
W_LO, W_HI = 7, 41
NW = W_HI - W_LO                        # 34 columns
# column classes:
#  simple  (w in [8,16)):  DVE STT arg = d*(d+2c), host descale exp(-a'c^2)
#  centered-DVE:           dm = d+c (tensor_scalar), arg = dm*dm (STT)
#  centered-ACT:           arg = Square(d + c) on ACT
SIMPLE_COLS = set(range(8, 16))
ACT_COLS = set(range(30, 41))
NGROUP = 3                              # pixel-chunk pipeline groups
GCHUNK = NCHUNK // NGROUP               # 256 chunks per group
DT = mybir.dt

_COMPILED = None  # cached (nc, meta)


def _emit_body(nc, tc, pool, pipe, psum_pool, x_ext, tbl_ext, emit_cols=True, emit_mm=True):
    xc = pool.tile([128, NCHUNK], DT.float32)
    nc.sync.dma_start(xc[:], x_ext[:])

    hi_i = pool.tile([128, NCHUNK], DT.int32)
    hi_f = pool.tile([128, NCHUNK], DT.float32)
    d_b = pool.tile([128, NCHUNK], DT.bfloat16)

    # hi = floor(x*15.9375) via RNE(x*15.9375 - 0.5) [HW converts RNE]
    nc.vector.tensor_scalar(hi_i[:], xc[:], 15.9375, -0.5,
                            mybir.AluOpType.mult, mybir.AluOpType.add)
    nc.vector.tensor_copy(hi_f[:], hi_i[:])
    # d = x*15.9375 - hi  in [0, 1]  (bf16)
    nc.vector.scalar_tensor_tensor(d_b[:], xc[:], 15.9375, hi_f[:],
                                   mybir.AluOpType.mult,
                                   mybir.AluOpType.subtract)

    oh = pool.tile([128, 16, NCHUNK], DT.float16)

    # dm[p, wi, cc] = d_b[p, cc] + (16 - w)/16, one 4x-mode ts_add per column
    dm = pool.tile([128, NW, NCHUNK], DT.bfloat16)
    if emit_cols:
        for wi in range(NW):
            nc.vector.tensor_scalar(dm[:, wi, :], d_b[:],
                                    (16.0 - (W_LO + wi)) / 16.0, None,
                                    mybir.AluOpType.add)

    acc = psum_pool.tile([16, UNITS * NW], DT.float32)

    for g in range(NGROUP):
        c0 = g * GCHUNK
        if emit_cols:
            # arg = dm*dm (bf16 2x), then weights = exp(-ALPHAP*arg) in fp16
            arg = pipe.tile([128, NW, GCHUNK], DT.bfloat16, tag="arg")
            bt_ = pipe.tile([128, NW, GCHUNK], DT.float16, tag="bwt")
            dmg = dm[:, :, c0:c0 + GCHUNK]
            nc.vector.tensor_tensor(arg[:], dmg, dmg, mybir.AluOpType.mult)
            nc.scalar.activation(bt_[:], arg[:],
                                 mybir.ActivationFunctionType.Exp,
                                 scale=-ALPHAP)
        else:
            bt_ = pipe.tile([128, NW, GCHUNK], DT.float16, tag="bwt")
        if g == 0:
            # one-hot of hi emitted here so it overlaps ACT's exp of group 0
            for m in range(16):
                nc.vector.tensor_scalar(oh[:, m, :], hi_f[:], float(m), None,
                                        mybir.AluOpType.is_equal)
        if emit_mm:
            for cc in range(GCHUNK):
                c = c0 + cc
                t, j = divmod(c, CHUNKS_PER_UNIT)
                nc.tensor.matmul(acc[:, t * NW:(t + 1) * NW],
                                 oh[:, :, c], bt_[:, :, cc],
                                 start=(j == 0),
                                 stop=(j == CHUNKS_PER_UNIT - 1))

    out_sb = pool.tile([16, UNITS * NW], DT.float32)
    if emit_mm:
        nc.vector.tensor_copy(out_sb[:], acc[:])
    else:
        nc.vector.tensor_copy(out_sb[:], oh[0:16, 0, 0:UNITS * NW])
    nc.sync.dma_start(tbl_ext[:], out_sb[:])


def _build(loop_n=1, emit_cols=True, emit_mm=True):
    nc = bacc.Bacc("TRN2", target_bir_lowering=False, debug=False,
                   num_devices=NCORES)
    x_ext = nc.declare_dram_parameter("xc", [128, NCHUNK], DT.float32,
                                      isOutput=False)
    tbl_ext = nc.declare_dram_parameter("table", [16, UNITS * NW], DT.float32,
                                        isOutput=True)

    with tile.TileContext(nc) as tc:
        with (
            tc.tile_pool(name="pool", bufs=1) as pool,
            tc.tile_pool(name="pipe", bufs=2) as pipe,
            tc.tile_pool(name="psum", bufs=1, space="PSUM") as psum_pool,
        ):
            if loop_n == 1:
                _emit_body(nc, tc, pool, pipe, psum_pool, x_ext, tbl_ext, emit_cols, emit_mm)
            else:
                engs = [mybir.EngineType.PE, mybir.EngineType.DVE,
                        mybir.EngineType.Activation, mybir.EngineType.SP,
                        mybir.EngineType.Pool]
                with tc.For_i(0, loop_n, 1, hint_engines=engs):
                    _emit_body(nc, tc, pool, pipe, psum_pool, x_ext, tbl_ext, emit_cols, emit_mm)

    nc.compile()
    return nc


def _get_compiled():
    global _COMPILED
    if _COMPILED is None:
        _COMPILED = _build()
    return _COMPILED


def _shard_x(x):
    """x (B,C,H,W) -> per-core [128, NCHUNK] arrays; element [p, 64t+j] =
    unit t, pixel 8192*core + 128*j + p."""
    xu = np.ascontiguousarray(x.reshape(UNITS, NPIX))
    shards = []
    for core in range(NCORES):
        sl = xu[:, core * PIX_PER_CORE:(core + 1) * PIX_PER_CORE]
        # (UNITS, 64, 128) -> (128, UNITS, 64)
        sl = sl.reshape(UNITS, CHUNKS_PER_UNIT, 128).transpose(2, 0, 1)
        shards.append(np.ascontiguousarray(sl.reshape(128, NCHUNK), np.float32))
    return shards


def _postprocess(tables):
    """tables: list of NCORES arrays [16, UNITS*NW] -> cdf (B, C, BINS) fp32."""
    tab = np.zeros((16, UNITS, NW), np.float64)
    for t in tables:
        tab += t.reshape(16, UNITS, NW).astype(np.float64)
    # descale simple-path columns by exp(-ALPHAP*c^2)
    ws = np.arange(W_LO, W_HI)
    cw = (16.0 - ws) / 16.0
    # all columns are centered (d+c)^2 form: no descale needed
    # fold: bin j = 16*(J-1) + w
    hist = np.zeros((UNITS, 16 + BINS + 48), np.float64)
    for J in range(16):
        hist[:, 16 * J + W_LO: 16 * J + W_HI] += tab[J, :, :]
    hist = hist[:, 16:16 + BINS]
    pdf = hist / (hist.sum(-1, keepdims=True) + 1e-6)
    cdf = np.cumsum(pdf, -1)
    return cdf.reshape(B, C, BINS).astype(np.float32)


def run_device(x, trace=False):
    nc = _get_compiled()
    in_maps = [{"xc": s} for s in _shard_x(np.asarray(x))]
    res = run_bass_kernel_spmd(nc, in_maps, list(range(NCORES)), trace=trace)
    tables = [res.results[i]["table"] for i in range(NCORES)]
    return tables, res


def kernel(x, centers):
    # centers is linspace(0,1,256) by construction; bin geometry is hardcoded.
    tables, _ = run_device(x)
    return _postprocess(tables)


if __name__ == "__main__":
    import jax, jax.numpy as jnp
    key = jax.random.key(0)
    k1, _ = jax.random.split(key)
    x = np.asarray(jax.random.uniform(k1, (B, C, H, W), dtype=jnp.float32))
    centers = np.linspace(0, 1, BINS, dtype=np.float32)
    out = kernel(x, centers)
    print("kernel output", out.shape, out.dtype, out[0, 0, :5], out[0, 0, -1])



# revision 7
# speedup vs baseline: 3.8531x; 3.8531x over previous
"""Trainium2 Bass kernel for nn_DifferentiableCDF (soft Gaussian histogram -> CDF).

Algorithm change vs the soft-binning baseline: the Gaussian soft-binning weight
exp(-(255x - j)^2 / 2.55^2) depends only on (u - j) with u = 255x, and each
pixel's total mass sum_j g(j - u) is (away from the [0,255] edges) a constant
independent of u.  Quantizing u -> m = round(u) therefore preserves per-pixel
mass exactly and perturbs the per-bin histogram only by ~N(0, h^2/12 * sum g'^2)
≈ 0.06% relative (verified 1.6e-4 end-to-end CDF error vs the fp64 reference).

So the device only computes a 256-bin COUNT histogram per (B,C) unit, and the
host applies the exact Gaussian spreading as a 33-tap float64 convolution,
normalizes, and cumsums (same scale of host work as the baseline's fold).

Device per core (98304 px as 768 chunks of 128):
  m = RNE(255x) in [0,255]; J = m>>4; r = m&15 (exact fp tricks).
  DVE builds 16-wide one-hots of J and r (bf16 4x tensor_scalar is_equal).
  TensorE: 8 chunks are packed per matmul: stationary = ohJ of 8 chunks side
  by side [128, 128] (full-width -> Fast Weight Load), moving = ohr of the
  same 8 chunks [128, 128]; out[8J+i, 8r+i'] accumulates in PSUM.  Only the
  slot-diagonal i=i' sub-blocks are meaningful (extracted on host); packing
  cuts 768 small matmuls down to 96 full-width ones.
"""
import sys
if "/opt/trn_rl_repo" not in sys.path:
    sys.path.insert(0, "/opt/trn_rl_repo")

import numpy as np
from concourse import bacc, tile
from concourse.bass_utils import run_bass_kernel_spmd
import concourse.mybir as mybir

# ---- problem constants (hardcoded per spec) ----
B, C, H, W = 4, 3, 256, 256
UNITS = B * C                  # 12 independent histograms
NPIX = H * W                   # 65536 pixels per unit
NCORES = 8
PIX_PER_CORE = NPIX // NCORES  # 8192 pixels per unit per core
CHUNKS_PER_UNIT = PIX_PER_CORE // 128  # 64
NCHUNK = UNITS * CHUNKS_PER_UNIT       # 768 chunks of 128 pixels
SIGMA = 0.01
BINS = 256
SIG_B = 255.0 * SIGMA                  # 2.55 bins: gaussian width in bin units
KTAP = 16                              # host conv halfwidth (g(16/2.55) ~ 6e-18)
NGROUP = 3                             # pipeline groups (4 units each)
GCHUNK = NCHUNK // NGROUP              # 256 chunks per group
PACK = 8                               # chunks per matmul
DT = mybir.dt

_COMPILED = None


def _emit_body(nc, tc, pool, pipe, psum_pool, x_ext, tbl_ext,
               emit_cols=True, emit_mm=True):
    xc = pool.tile([128, NCHUNK], DT.float32)
    nc.sync.dma_start(xc[:], x_ext[:])

    m_i = pool.tile([128, NCHUNK], DT.int32)
    m_f = pool.tile([128, NCHUNK], DT.float32)
    J_i = pool.tile([128, NCHUNK], DT.int32)
    J_f = pool.tile([128, NCHUNK], DT.float32)
    J_b = pool.tile([128, NCHUNK // PACK, PACK], DT.bfloat16)
    r_b = pool.tile([128, NCHUNK // PACK, PACK], DT.bfloat16)

    # m = RNE(255*x) via int32-convert; exact fp32 (255x <= 255).
    nc.vector.tensor_scalar(m_i[:], xc[:], 255.0, None, mybir.AluOpType.mult)
    nc.vector.tensor_copy(m_f[:], m_i[:])
    # J = floor(m/16) = RNE(m/16 - 15/32): m/16 lies on a 1/16 grid, so the
    # offset keeps every value >= 1/32 away from a rounding boundary.
    nc.vector.tensor_scalar(J_i[:], m_f[:], 0.0625, -0.46875,
                            mybir.AluOpType.mult, mybir.AluOpType.add)
    nc.vector.tensor_copy(J_f[:], J_i[:])
    nc.vector.tensor_copy(J_b[:], J_i[:])
    # r = m - 16J in [0,16)
    nc.vector.scalar_tensor_tensor(r_b[:], J_f[:], -16.0, m_f[:],
                                   mybir.AluOpType.mult, mybir.AluOpType.add)

    accs = [psum_pool.tile([128, 4 * PACK * 16], DT.float32, name=f"acc{g}")
            for g in range(NGROUP)]
    out_sb = pool.tile([128, NGROUP * 512], DT.float32)

    GPACK = GCHUNK // PACK  # 32 packs per group
    for g in range(NGROUP):
        p0 = g * GPACK
        # pack-local one-hots: ohJ[:, p, 8v + i] = [J(chunk 8(p0+p)+i) == v]
        ohJ = pipe.tile([128, GPACK, 16 * PACK], DT.bfloat16, tag="ohJ")
        ohr = pipe.tile([128, GPACK, 16 * PACK], DT.bfloat16, tag="ohr")
        if emit_cols:
            for v in range(16):
                nc.vector.tensor_scalar(ohJ[:, :, v * PACK:(v + 1) * PACK],
                                        J_b[:, p0:p0 + GPACK, :],
                                        float(v), None,
                                        mybir.AluOpType.is_equal)
                nc.vector.tensor_scalar(ohr[:, :, v * PACK:(v + 1) * PACK],
                                        r_b[:, p0:p0 + GPACK, :],
                                        float(v), None,
                                        mybir.AluOpType.is_equal)
        if emit_mm:
            npk = CHUNKS_PER_UNIT // PACK  # 8 packs per unit
            for uu in range(4):  # 4 units per group
                for q in range(npk):
                    pl = uu * npk + q
                    nc.tensor.matmul(accs[g][:, uu * 128:(uu + 1) * 128],
                                     ohJ[:, pl, :],
                                     ohr[:, pl, :],
                                     start=(q == 0), stop=(q == npk - 1))
            nc.scalar.copy(out_sb[:, g * 512:(g + 1) * 512], accs[g][:])
            nc.sync.dma_start(tbl_ext[:, g * 512:(g + 1) * 512],
                              out_sb[:, g * 512:(g + 1) * 512])
        else:
            nc.scalar.copy(out_sb[:, g * 512:(g + 1) * 512], accs[g][:])
            nc.sync.dma_start(tbl_ext[:, g * 512:(g + 1) * 512],
                              out_sb[:, g * 512:(g + 1) * 512])


def _build(loop_n=1, emit_cols=True, emit_mm=True):
    nc = bacc.Bacc("TRN2", target_bir_lowering=False, debug=False,
                   num_devices=NCORES)
    x_ext = nc.declare_dram_parameter("xc", [128, NCHUNK], DT.float32,
                                      isOutput=False)
    tbl_ext = nc.declare_dram_parameter("table", [128, NGROUP * 512],
                                        DT.float32, isOutput=True)

    with tile.TileContext(nc) as tc:
        with (
            tc.tile_pool(name="pool", bufs=1) as pool,
            tc.tile_pool(name="pipe", bufs=2) as pipe,
            tc.tile_pool(name="psum", bufs=1, space="PSUM") as psum_pool,
        ):
            if loop_n == 1:
                _emit_body(nc, tc, pool, pipe, psum_pool, x_ext, tbl_ext,
                           emit_cols, emit_mm)
            else:
                engs = [mybir.EngineType.PE, mybir.EngineType.DVE,
                        mybir.EngineType.Activation, mybir.EngineType.SP,
                        mybir.EngineType.Pool]
                with tc.For_i(0, loop_n, 1, hint_engines=engs):
                    _emit_body(nc, tc, pool, pipe, psum_pool, x_ext, tbl_ext,
                               emit_cols, emit_mm)

    nc.compile()
    return nc


def _get_compiled():
    global _COMPILED
    if _COMPILED is None:
        _COMPILED = _build()
    return _COMPILED


def _shard_x(x):
    """x (B,C,H,W) -> per-core [128, NCHUNK] arrays; element [p, 64t+j] =
    unit t, pixel 8192*core + 128*j + p."""
    xu = np.ascontiguousarray(x.reshape(UNITS, NPIX))
    shards = []
    for core in range(NCORES):
        sl = xu[:, core * PIX_PER_CORE:(core + 1) * PIX_PER_CORE]
        sl = sl.reshape(UNITS, CHUNKS_PER_UNIT, 128).transpose(2, 0, 1)
        shards.append(np.ascontiguousarray(sl.reshape(128, NCHUNK), np.float32))
    return shards


def _postprocess(tables):
    """tables: list of NCORES arrays [128, 1536] -> cdf (B, C, BINS) fp32."""
    cnt = np.zeros((UNITS, 16, 16), np.float64)   # [unit, J, r]
    for t in tables:
        # rows = (J:16, i:8); cols = (g:3, uu:4, r:16, i':8); diag i==i'
        t6 = t.reshape(16, 8, NGROUP, 4, 16, 8).astype(np.float64)
        cnt += np.einsum('jiguri->gujr', t6).reshape(UNITS, 16, 16)
    count = cnt.reshape(UNITS, BINS)              # bin m = 16J + r
    ks = np.arange(-KTAP, KTAP + 1)
    g = np.exp(-(ks / SIG_B) ** 2)
    hist = np.zeros((UNITS, BINS), np.float64)
    for i, k in enumerate(ks):
        lo, hi = max(0, k), min(BINS, BINS + k)
        hist[:, lo:hi] += g[i] * count[:, lo - k:hi - k]
    pdf = hist / (hist.sum(-1, keepdims=True) + 1e-6)
    cdf = np.cumsum(pdf, -1)
    return cdf.reshape(B, C, BINS).astype(np.float32)


def run_device(x, trace=False):
    nc = _get_compiled()
    in_maps = [{"xc": s} for s in _shard_x(np.asarray(x))]
    res = run_bass_kernel_spmd(nc, in_maps, list(range(NCORES)), trace=trace)
    tables = [res.results[i]["table"] for i in range(NCORES)]
    return tables, res


def kernel(x, centers):
    # centers is linspace(0,1,256) by construction; bin geometry is hardcoded.
    tables, _ = run_device(x)
    return _postprocess(tables)


if __name__ == "__main__":
    import jax, jax.numpy as jnp
    key = jax.random.key(0)
    k1, _ = jax.random.split(key)
    x = np.asarray(jax.random.uniform(k1, (B, C, H, W), dtype=jnp.float32))
    centers = np.linspace(0, 1, BINS, dtype=np.float32)
    out = kernel(x, centers)
    print("kernel output", out.shape, out.dtype, out[0, 0, :5], out[0, 0, -1])
